# revision 10
# baseline (speedup 1.0000x reference)
"""Trainium2 Bass kernel for a 4-layer NeRF-style MLP.

    y = relu(relu(relu(x@W1.T+b1)@W2.T+b2)@W3.T+b3)@W4.T+b4
    x: [1048576, 6] fp32 -> y: [1048576, 4] fp32

Strategy: pure data parallel over 8 NeuronCores (131072 rows each).
On-device layout keeps features on SBUF partitions and rows on the free
dim, so every layer's PSUM output is directly the next layer's matmul
rhs -- no transposes anywhere.

The entire on-device datapath is bf16 (x, W1..W4, h1..h3): every
LDWEIGHTS gets Fast Weight Load (2 cols/cycle), the moving operands
halve their SBUF read bandwidth, and the PE instruction stream is dense
enough to keep the HAM clock-gate at K=8/8 (2.4 GHz).  PSUM stays f32.

Per core, rows are processed in groups of 4 chunks x 512 rows, split
into two independent half-group chains (a: chunks 0-1, b: chunks 2-3),
each owning a 2-bank PSUM tile.  With pool bufs=2, four half-chains are
in flight, which hides each PSUM->SBUF eviction's latency behind the
other chains' matmuls.

  - layer 1 (K=6+1): the 4 chunks are packed into the four 32-row PE
    groups (tile_position row packing) and run concurrently; the bias is
    folded into the matmul via a constant ones-row in x (K=7).
  - layers 2/3 (K=128): one matmul per chunk, bf16.
  - layer 4 is computed transposed (h3-slice stationary, W4.T moving,
    N=4); the group's output is a dense [128, 32] PSUM block per half,
    so its eviction is nearly free.  b4 is added on the host.
  - evictions are fused bias+ReLU ops; the a/b chain <-> ScalarE/VectorE
    assignment alternates by group parity.
  - all weights ship as one packed [128, 388] bf16 input plus a tiny
    [128, 2] f32 bias input.
"""

import numpy as np

N = 1048576
CORES = 8
R = N // CORES            # rows per core
CHUNK = 512               # rows per matmul (one PSUM bank of fp32)
GPC = 4                   # chunks per group
GROUPS = R // (CHUNK * GPC)   # 64
GW = GPC * CHUNK          # 2048 columns per group
REPEAT = 1                # times to run the whole compute body (bench only)
HEATER = True             # keep-HAM-warm dummy matmul per group

_CACHE = {}


def _build():
    import concourse.bacc as bacc
    import concourse.mybir as mybir
    import concourse.tile as tile
    from concourse.compiler_utils import get_compiler_flags, set_compiler_flags

    # The environment's default backend options ship --enable-ldw-opt=false
    # (a blanket workaround for an FWL hang that only affects fp32-HI
    # matmuls).  Every matmul in this kernel is bf16, so Fast Weight Load
    # is safe -- and halves every LDWEIGHTS (128 cols: ~107ns -> ~55ns).
    set_compiler_flags(
        [
            f.replace("--enable-ldw-opt=false", "--enable-ldw-opt=true")
            for f in get_compiler_flags()
        ]
    )

    f32 = mybir.dt.float32
    bf16 = mybir.dt.bfloat16
    Relu = mybir.ActivationFunctionType.Relu
    op_add = mybir.AluOpType.add
    op_max = mybir.AluOpType.max

    nc = bacc.Bacc("TRN2", target_bir_lowering=False, debug=False)

    xin = nc.dram_tensor(
        "xin", [GROUPS // 4, GPC, 7, 4 * CHUNK], bf16, kind="ExternalInput"
    ).ap()
    wpack = nc.dram_tensor(
        "wpack", [128, 388], bf16, kind="ExternalInput"
    ).ap()  # w1(+b1) | w2 | w3 | w4
    bpack = nc.dram_tensor(
        "bpack", [128, 2], f32, kind="ExternalInput"
    ).ap()  # b2 | b3
    yout = nc.dram_tensor(
        "yout", [GROUPS // 4, 2, 128, 128], bf16, kind="ExternalOutput"
    ).ap()

    with tile.TileContext(nc) as tc:
        with (
            tc.tile_pool(name="const", bufs=1) as cpool,
            tc.tile_pool(name="x", bufs=4) as xpool,
            tc.tile_pool(name="h", bufs=6) as hpool,
            tc.tile_pool(name="o", bufs=4) as opool,
            tc.tile_pool(name="psum", bufs=2, space="PSUM") as ppool,
        ):
            wps = cpool.tile([128, 388], bf16, tag="wp")
            nc.sync.dma_start(out=wps[:], in_=wpack)
            bps = cpool.tile([128, 2], f32, tag="bp")
            nc.sync.dma_start(out=bps[:], in_=bpack)
            w1s = wps[:, 0:128]
            w2s = wps[:, 128:256]
            w3s = wps[:, 256:384]
            w4s = wps[:, 384:388]
            b2s = bps[:, 0:1]
            b3s = bps[:, 1:2]

            w1r = w1s.rearrange("(a b) c -> a b c", b=32)

            HW = GW // 2  # 1024 columns: each half-group (2 chunks)
            st = {}       # per-group in-flight tiles
            xts = {}      # x tile per 4-group block
            oab = {}      # output accumulation tiles per 4-group block

            def evict_relu(use_act, out_ap, in_ap, bias_ap):
                """bias+ReLU PSUM->SBUF eviction on either engine."""
                if use_act:
                    if bias_ap is None:
                        nc.scalar.activation(out_ap, in_ap, Relu)
                    else:
                        nc.scalar.activation(out_ap, in_ap, Relu, bias=bias_ap)
                elif bias_ap is None:
                    nc.vector.tensor_scalar(
                        out=out_ap,
                        in0=in_ap,
                        scalar1=0.0,
                        scalar2=None,
                        op0=op_max,
                    )
                else:
                    nc.vector.tensor_scalar(
                        out=out_ap,
                        in0=in_ap,
                        scalar1=bias_ap,
                        scalar2=0.0,
                        op0=op_add,
                        op1=op_max,
                    )

            def evict_copy(use_act, out_ap, in_ap):
                if use_act:
                    nc.scalar.activation(
                        out_ap, in_ap, mybir.ActivationFunctionType.Copy
                    )
                else:
                    nc.vector.tensor_copy(out=out_ap, in_=in_ap)

            def load_x(blk):
                """DMA one 4-group block of x into SBUF."""
                if blk >= GROUPS // 4 or blk in xts:
                    return
                xt = xpool.tile([128, 4 * CHUNK], bf16, tag="x")
                xtr = xt.rearrange("(a b) c -> a b c", b=32)
                for c in range(GPC):
                    nc.sync.dma_start(out=xtr[c, 0:7, :], in_=xin[blk, c])
                xts[blk] = xtr

            def use_act(g, half):
                # alternate engine<->half-chain by group parity to balance
                return (g % 2 == 0) == (half == 0)

            # engine assignment per layer: ScalarE is faster per tile
            # (1.2 GHz vs 0.96), so it gets both h3 evictions (shortening
            # layer 4's critical path) plus both tiny L4 output copies;
            # VectorE gets both h2; h1 alternates to keep 3/3 tiles each.

            def l1(g, half):
                """layer-1 matmuls + L1 eviction for one half-group (x is
                prefetched a block ahead so L1 never waits on the DMA)."""
                q = g % 4
                if half == 0 and q == 0:
                    load_x(g // 4)      # no-op except for block 0
                    load_x(g // 4 + 1)  # prefetch next block
                xtr = xts[g // 4]
                p = ppool.tile([128, HW], f32, tag=f"p{half}")
                for cc in range(2):
                    c = 2 * half + cc
                    nc.tensor.matmul(
                        p[:, cc * CHUNK : (cc + 1) * CHUNK],
                        lhsT=w1r[c, 0:7, :],
                        rhs=xtr[c, 0:7, q * CHUNK : (q + 1) * CHUNK],
                        start=True,
                        stop=True,
                        tile_position=(32 * c, 0),
                    )
                h = hpool.tile([128, HW], bf16, tag=f"h1{half}")
                evict_relu(use_act(g, half), h[:, :], p[:, :], None)
                st[(g, half)] = {"p": p, "h1": h}

            def l2(g, half):
                s = st[(g, half)]
                for cc in range(2):
                    nc.tensor.matmul(
                        s["p"][:, cc * CHUNK : (cc + 1) * CHUNK],
                        lhsT=w2s,
                        rhs=s["h1"][:, cc * CHUNK : (cc + 1) * CHUNK],
                        start=True,
                        stop=True,
                    )
                h = hpool.tile([128, HW], bf16, tag=f"h2{half}")
                evict_relu(False, h[:, :], s["p"][:, :], b2s)
                s["h2"] = h

            def l3(g, half):
                s = st[(g, half)]
                for cc in range(2):
                    nc.tensor.matmul(
                        s["p"][:, cc * CHUNK : (cc + 1) * CHUNK],
                        lhsT=w3s,
                        rhs=s["h2"][:, cc * CHUNK : (cc + 1) * CHUNK],
                        start=True,
                        stop=True,
                    )
                h = hpool.tile([128, HW], bf16, tag=f"h3{half}")
                evict_relu(True, h[:, :], s["p"][:, :], b3s)
                s["h3"] = h

            def l4(g, half):
                """layer 4 (transposed, bf16 FWL), output copy + DMA."""
                s = st.pop((g, half))
                q = g % 4
                for sl in range(8):
                    nc.tensor.matmul(
                        s["p"][:, 4 * sl : 4 * sl + 4],
                        lhsT=s["h3"][:, 128 * sl : 128 * sl + 128],
                        rhs=w4s[:, :],
                        start=True,
                        stop=True,
                        skip_group_check=True,
                    )
                if q == 0 and half == 0:
                    ota = opool.tile([128, 128], bf16, tag="oa")
                    otb = opool.tile([128, 128], bf16, tag="ob")
                    oab[g // 4] = (ota, otb)
                ot = oab[g // 4][half]
                evict_copy(True, ot[:, 32 * q : 32 * q + 32], s["p"][:, 0:32])
                if HEATER and half == 1:
                    # dummy matmul into the already-evicted region of this
                    # PSUM tile: real PE-array streaming activity in the
                    # otherwise LDW-heavy L4 phase, keeping the HAM
                    # clock-gate at K=8/8.  Output is garbage, never read;
                    # the tile recycles to L1 right after (WAW-serialized).
                    nc.tensor.matmul(
                        s["p"][:, 0:384],
                        lhsT=w2s,
                        rhs=wps[:, 0:384],
                        start=True,
                        stop=True,
                        skip_group_check=True,
                    )
                if q == 3 and half == 1:
                    ota, otb = oab[g // 4]
                    nc.sync.dma_start(out=yout[g // 4, 0], in_=ota[:])
                    nc.sync.dma_start(out=yout[g // 4, 1], in_=otb[:])
                    del oab[g // 4], xts[g // 4]

            # software pipeline over groups at half-group granularity: the
            # independent half-chains give the scheduler material to fill
            # every eviction wait with another chain's matmuls.
            for gg in [g for _ in range(REPEAT) for g in range(GROUPS + 1)]:
                if gg < GROUPS:
                    l1(gg, 0)
                    l1(gg, 1)
                if gg >= 1:
                    l3(gg - 1, 0)
                    l3(gg - 1, 1)
                if gg < GROUPS:
                    l2(gg, 0)
                    l2(gg, 1)
                if gg >= 1:
                    l4(gg - 1, 0)
                    l4(gg - 1, 1)

    nc.compile()
    return nc


def _prep_in_maps(x, W1, b1, W2, b2, W3, b3, W4, b4):
    import ml_dtypes

    bf16 = ml_dtypes.bfloat16
    x = np.asarray(x, dtype=np.float32)

    wp = np.zeros((128, 388), bf16)
    W1T = np.asarray(W1, np.float32).T  # [6, 128]
    for g in range(GPC):
        wp[32 * g : 32 * g + 6, 0:128] = W1T.astype(bf16)
        wp[32 * g + 6, 0:128] = np.asarray(b1, np.float32).astype(bf16)
    wp[:, 128:256] = np.asarray(W2, np.float32).T.astype(bf16)
    wp[:, 256:384] = np.asarray(W3, np.float32).T.astype(bf16)
    wp[:, 384:388] = np.asarray(W4, np.float32).T.astype(bf16)

    bp = np.zeros((128, 2), np.float32)
    bp[:, 0] = np.asarray(b2, np.float32)
    bp[:, 1] = np.asarray(b3, np.float32)

    in_maps = []
    for c in range(CORES):
        xc = x[c * R : (c + 1) * R]  # [R, 6]
        # xin[xg, g, k, q*CHUNK + j] = xc[((xg*4 + q)*GPC + g)*CHUNK + j, k]
        xr = xc.reshape(GROUPS // 4, 4, GPC, CHUNK, 6).transpose(0, 2, 4, 1, 3)
        xr = xr.reshape(GROUPS // 4, GPC, 6, 4 * CHUNK)
        xi = np.empty((GROUPS // 4, GPC, 7, 4 * CHUNK), bf16)
        xi[:, :, 0:6, :] = xr.astype(bf16)
        xi[:, :, 6, :] = 1.0
        in_maps.append({"xin": xi, "wpack": wp, "bpack": bp})
    return in_maps


def _execute(in_maps, trace=False):
    from concourse.bass_utils import run_bass_kernel_spmd

    if "nc" not in _CACHE:
        _CACHE["nc"] = _build()
    return run_bass_kernel_spmd(
        _CACHE["nc"], in_maps, list(range(CORES)), trace=trace
    )


def bench(in_maps, iters=20):
    """Measure the per-iteration device-side execution time of the kernel.

    The NeuronCores are reached through an axon tunnel whose host<->device
    round-trip latency is ~60 ms — three orders of magnitude above the
    kernel itself — so timing one synchronous dispatch measures the
    network, not the hardware.  Instead we enqueue N dispatches
    back-to-back (device-resident inputs, one final block_until_ready) so
    consecutive NEFF executions pipeline on-device, and recover the
    marginal per-iteration cost as the slope between a short and a long
    pipelined batch: slope = (T(N2) - T(N1)) / (N2 - N1).  The one-time
    tunnel round trip cancels in the difference.  Batches are repeated
    interleaved and min-aggregated to reject one-sided scheduling noise.

    Returns [slope_seconds] (list, for min() compatibility).
    """
    import time

    import jax
    from jax.experimental.shard_map import shard_map
    from jax.sharding import Mesh, NamedSharding, PartitionSpec

    import concourse.mybir as mybir
    from concourse import bass2jax

    if "nc" not in _CACHE:
        _CACHE["nc"] = _build()
    nc = _CACHE["nc"]
    bass2jax.install_neuronx_cc_hook()

    in_names, out_names, out_avals = [], [], []
    for alloc in nc.m.functions[0].allocations:
        if not isinstance(alloc, mybir.MemoryLocationSet):
            continue
        name = alloc.memorylocations[0].name
        pid = nc.partition_id_tensor.name if nc.partition_id_tensor else None
        if alloc.kind == "ExternalInput":
            if name != pid:
                in_names.append(name)
        elif alloc.kind == "ExternalOutput":
            out_names.append(name)
            out_avals.append(
                jax.core.ShapedArray(
                    tuple(alloc.tensor_shape), mybir.dt.np(alloc.dtype)
                )
            )
    n_params = len(in_names)
    all_names = tuple(in_names + out_names)

    def _body(*args):
        operands = list(args)
        if nc.partition_id_tensor is not None:
            operands.append(bass2jax.partition_id_tensor())
        outs = bass2jax._bass_exec_p.bind(
            *operands,
            out_avals=tuple(out_avals),
            in_names=all_names
            + ((nc.partition_id_tensor.name,) if nc.partition_id_tensor else ()),
            out_names=tuple(out_names),
            lowering_input_output_aliases=(),
            sim_require_finite=True,
            sim_require_nnan=True,
            nc=nc,
        )
        return tuple(outs)

    devices = jax.devices()[:CORES]
    mesh = Mesh(np.asarray(devices), ("core",))
    in_specs = (PartitionSpec("core"),) * (n_params + len(out_names))
    out_specs = (PartitionSpec("core"),) * len(out_names)
    sm = shard_map(
        _body, mesh=mesh, in_specs=in_specs, out_specs=out_specs, check_rep=False
    )

    concat_in = [
        np.concatenate([np.asarray(in_maps[c][n]) for c in range(CORES)], axis=0)
        for n in in_names
    ]
    zeros = [
        np.zeros((CORES * av.shape[0], *av.shape[1:]), av.dtype) for av in out_avals
    ]
    sh = NamedSharding(mesh, PartitionSpec("core"))
    dev_in = [jax.device_put(a, sh) for a in concat_in]
    dev_zeros = [jax.device_put(z, sh) for z in zeros]

    fn = bass2jax.fast_dispatch_compile(
        lambda: jax.jit(sm, keep_unused=True).lower(*dev_in, *dev_zeros).compile()
    )

    def batch(n):
        t0 = time.perf_counter()
        out = None
        for _ in range(n):
            out = fn(*dev_in, *dev_zeros)
        jax.block_until_ready(out)
        return time.perf_counter() - t0

    batch(2)  # warmup
    n1, n2, reps = 10, 100, max(8, iters // 3)
    t1s, t2s = [], []
    for _ in range(reps):
        t1s.append(batch(n1))
        t2s.append(batch(n2))
    slope = (min(t2s) - min(t1s)) / (n2 - n1)
    slope = max(slope, 1e-9)
    print(
        f"bench: T({n1}) {[round(t * 1e3, 2) for t in t1s]} ms, "
        f"T({n2}) {[round(t * 1e3, 2) for t in t2s]} ms"
    )
    return [slope]


def kernel(**inputs):
    in_maps = _prep_in_maps(
        inputs["x"],
        inputs["W1"],
        inputs["b1"],
        inputs["W2"],
        inputs["b2"],
        inputs["W3"],
        inputs["b3"],
        inputs["W4"],
        inputs["b4"],
    )
    results = _execute(in_maps).results
    outs = []
    for c in range(CORES):
        # yout dims: (xg, half, p, (q, s4, k)); group = xg*4 + q,
        # row = group*2048 + half*1024 + s4*128 + p
        yo = (
            np.asarray(results[c]["yout"])
            .astype(np.float32)
            .reshape(GROUPS // 4, 2, 128, 4, 8, 4)
        )
        outs.append(yo.transpose(0, 3, 1, 4, 2, 5).reshape(R, 4))
    y = np.concatenate(outs, axis=0)
    y += np.asarray(inputs["b4"], np.float32)  # layer-4 bias, added on host
    return np.ascontiguousarray(y.astype(np.float32))


# revision 11
# speedup vs baseline: 1.0562x; 1.0562x over previous
"""Trainium2 Bass kernel for a 4-layer NeRF-style MLP.

    y = relu(relu(relu(x@W1.T+b1)@W2.T+b2)@W3.T+b3)@W4.T+b4
    x: [1048576, 6] fp32 -> y: [1048576, 4] fp32

Strategy: pure data parallel over 8 NeuronCores (131072 rows each).
On-device layout keeps features on SBUF partitions and rows on the free
dim, so every layer's PSUM output is directly the next layer's matmul
rhs -- no transposes anywhere.

The entire on-device datapath is bf16 (x, W1..W4, h1..h3): every
LDWEIGHTS gets Fast Weight Load (2 cols/cycle), the moving operands
halve their SBUF read bandwidth, and the PE instruction stream is dense
enough to keep the HAM clock-gate at K=8/8 (2.4 GHz).  PSUM stays f32.

Per core, rows are processed in groups of 4 chunks x 512 rows, split
into two independent half-group chains (a: chunks 0-1, b: chunks 2-3),
each owning a 2-bank PSUM tile.  With pool bufs=2, four half-chains are
in flight, which hides each PSUM->SBUF eviction's latency behind the
other chains' matmuls.

  - layer 1 (K=6+1): the 4 chunks are packed into the four 32-row PE
    groups (tile_position row packing) and run concurrently; the bias is
    folded into the matmul via a constant ones-row in x (K=7).
  - layers 2/3 (K=128): one matmul per chunk, bf16.
  - layer 4 is computed transposed (h3-slice stationary, W4.T moving,
    N=4); the group's output is a dense [128, 32] PSUM block per half,
    so its eviction is nearly free.  b4 is added on the host.
  - evictions are fused bias+ReLU ops; the a/b chain <-> ScalarE/VectorE
    assignment alternates by group parity.
  - all weights ship as one packed [128, 388] bf16 input plus a tiny
    [128, 2] f32 bias input.
"""

import numpy as np

N = 1048576
CORES = 8
R = N // CORES            # rows per core
CHUNK = 512               # rows per matmul (one PSUM bank of fp32)
GPC = 4                   # chunks per group
GROUPS = R // (CHUNK * GPC)   # 64
GW = GPC * CHUNK          # 2048 columns per group
REPEAT = 1                # times to run the whole compute body (bench only)
HEATER = True             # keep-HAM-warm dummy matmul per group

_CACHE = {}


def _build():
    import concourse.bacc as bacc
    import concourse.mybir as mybir
    import concourse.tile as tile
    from concourse.compiler_utils import get_compiler_flags, set_compiler_flags

    # The environment's default backend options ship --enable-ldw-opt=false
    # (a blanket workaround for an FWL hang that only affects fp32-HI
    # matmuls).  Every matmul in this kernel is bf16, so Fast Weight Load
    # is safe -- and halves every LDWEIGHTS (128 cols: ~107ns -> ~55ns).
    set_compiler_flags(
        [
            f.replace("--enable-ldw-opt=false", "--enable-ldw-opt=true")
            for f in get_compiler_flags()
        ]
    )

    f32 = mybir.dt.float32
    bf16 = mybir.dt.bfloat16
    Relu = mybir.ActivationFunctionType.Relu
    op_add = mybir.AluOpType.add
    op_max = mybir.AluOpType.max

    nc = bacc.Bacc("TRN2", target_bir_lowering=False, debug=False)

    xin = nc.dram_tensor(
        "xin", [GROUPS // 4, GPC, 7, 4 * CHUNK], bf16, kind="ExternalInput"
    ).ap()
    wpack = nc.dram_tensor(
        "wpack", [128, 388], bf16, kind="ExternalInput"
    ).ap()  # w1(+b1) | w2 | w3 | w4
    bpack = nc.dram_tensor(
        "bpack", [128, 2], f32, kind="ExternalInput"
    ).ap()  # b2 | b3
    yout = nc.dram_tensor(
        "yout", [GROUPS // 4, 2, 128, 128], bf16, kind="ExternalOutput"
    ).ap()

    with tile.TileContext(nc) as tc:
        with (
            tc.tile_pool(name="const", bufs=1) as cpool,
            tc.tile_pool(name="x", bufs=4) as xpool,
            tc.tile_pool(name="h", bufs=6) as hpool,
            tc.tile_pool(name="o", bufs=4) as opool,
            tc.tile_pool(name="psum", bufs=2, space="PSUM") as ppool,
        ):
            wps = cpool.tile([128, 388], bf16, tag="wp")
            nc.sync.dma_start(out=wps[:], in_=wpack)
            bps = cpool.tile([128, 2], f32, tag="bp")
            nc.sync.dma_start(out=bps[:], in_=bpack)
            w1s = wps[:, 0:128]
            w2s = wps[:, 128:256]
            w3s = wps[:, 256:384]
            w4s = wps[:, 384:388]
            b2s = bps[:, 0:1]
            b3s = bps[:, 1:2]

            w1r = w1s.rearrange("(a b) c -> a b c", b=32)

            HW = GW // 2  # 1024 columns: each half-group (2 chunks)
            st = {}       # per-group in-flight tiles
            xts = {}      # x tile per 4-group block
            oab = {}      # output accumulation tiles per 4-group block

            def evict_relu(use_act, out_ap, in_ap, bias_ap):
                """bias+ReLU PSUM->SBUF eviction on either engine."""
                if use_act:
                    if bias_ap is None:
                        nc.scalar.activation(out_ap, in_ap, Relu)
                    else:
                        nc.scalar.activation(out_ap, in_ap, Relu, bias=bias_ap)
                elif bias_ap is None:
                    nc.vector.tensor_scalar(
                        out=out_ap,
                        in0=in_ap,
                        scalar1=0.0,
                        scalar2=None,
                        op0=op_max,
                    )
                else:
                    nc.vector.tensor_scalar(
                        out=out_ap,
                        in0=in_ap,
                        scalar1=bias_ap,
                        scalar2=0.0,
                        op0=op_add,
                        op1=op_max,
                    )

            def evict_copy(use_act, out_ap, in_ap):
                if use_act:
                    nc.scalar.activation(
                        out_ap, in_ap, mybir.ActivationFunctionType.Copy
                    )
                else:
                    nc.vector.tensor_copy(out=out_ap, in_=in_ap)

            def load_x(blk):
                """DMA one 4-group block of x into SBUF."""
                if blk >= GROUPS // 4 or blk in xts:
                    return
                xt = xpool.tile([128, 4 * CHUNK], bf16, tag="x")
                xtr = xt.rearrange("(a b) c -> a b c", b=32)
                for c in range(GPC):
                    nc.sync.dma_start(out=xtr[c, 0:7, :], in_=xin[blk, c])
                xts[blk] = xtr

            def use_act(g, half):
                # alternate engine<->half-chain by group parity to balance
                return (g % 2 == 0) == (half == 0)

            def l1(g, half):
                """layer-1 matmuls + L1 eviction for one half-group (x is
                prefetched a block ahead so L1 never waits on the DMA)."""
                q = g % 4
                if half == 0 and q == 0:
                    load_x(g // 4)      # no-op except for block 0
                    load_x(g // 4 + 1)  # prefetch next block
                xtr = xts[g // 4]
                p = ppool.tile([128, HW], f32, tag=f"p{half}")
                for cc in range(2):
                    c = 2 * half + cc
                    nc.tensor.matmul(
                        p[:, cc * CHUNK : (cc + 1) * CHUNK],
                        lhsT=w1r[c, 0:7, :],
                        rhs=xtr[c, 0:7, q * CHUNK : (q + 1) * CHUNK],
                        start=True,
                        stop=True,
                        tile_position=(32 * c, 0),
                    )
                h = hpool.tile([128, HW], bf16, tag=f"h1{half}")
                evict_relu(use_act(g, half), h[:, :], p[:, :], None)
                st[(g, half)] = {"p": p, "h1": h}

            def l2(g, half):
                s = st[(g, half)]
                for cc in range(2):
                    nc.tensor.matmul(
                        s["p"][:, cc * CHUNK : (cc + 1) * CHUNK],
                        lhsT=w2s,
                        rhs=s["h1"][:, cc * CHUNK : (cc + 1) * CHUNK],
                        start=True,
                        stop=True,
                    )
                h = hpool.tile([128, HW], bf16, tag=f"h2{half}")
                evict_relu(use_act(g, half), h[:, :], s["p"][:, :], b2s)
                s["h2"] = h

            def l3(g, half):
                s = st[(g, half)]
                for cc in range(2):
                    nc.tensor.matmul(
                        s["p"][:, cc * CHUNK : (cc + 1) * CHUNK],
                        lhsT=w3s,
                        rhs=s["h2"][:, cc * CHUNK : (cc + 1) * CHUNK],
                        start=True,
                        stop=True,
                    )
                h = hpool.tile([128, HW], bf16, tag=f"h3{half}")
                evict_relu(use_act(g, half), h[:, :], s["p"][:, :], b3s)
                s["h3"] = h

            def l4(g, half):
                """layer 4 (transposed, bf16 FWL), output copy + DMA."""
                s = st.pop((g, half))
                q = g % 4
                for sl in range(8):
                    nc.tensor.matmul(
                        s["p"][:, 4 * sl : 4 * sl + 4],
                        lhsT=s["h3"][:, 128 * sl : 128 * sl + 128],
                        rhs=w4s[:, :],
                        start=True,
                        stop=True,
                        skip_group_check=True,
                    )
                if q == 0 and half == 0:
                    ota = opool.tile([128, 128], bf16, tag="oa")
                    otb = opool.tile([128, 128], bf16, tag="ob")
                    oab[g // 4] = (ota, otb)
                ot = oab[g // 4][half]
                evict_copy(use_act(g, half), ot[:, 32 * q : 32 * q + 32], s["p"][:, 0:32])
                if HEATER and half == 1:
                    # dummy matmul into the already-evicted region of this
                    # PSUM tile: real PE-array streaming activity in the
                    # otherwise LDW-heavy L4 phase, keeping the HAM
                    # clock-gate at K=8/8.  Output is garbage, never read;
                    # the tile recycles to L1 right after (WAW-serialized).
                    nc.tensor.matmul(
                        s["p"][:, 0:384],
                        lhsT=w2s,
                        rhs=wps[:, 0:384],
                        start=True,
                        stop=True,
                        skip_group_check=True,
                    )
                if q == 3 and half == 1:
                    ota, otb = oab[g // 4]
                    nc.sync.dma_start(out=yout[g // 4, 0], in_=ota[:])
                    nc.sync.dma_start(out=yout[g // 4, 1], in_=otb[:])
                    del oab[g // 4], xts[g // 4]

            # software pipeline over groups at half-group granularity: the
            # independent half-chains give the scheduler material to fill
            # every eviction wait with another chain's matmuls.
            for gg in [g for _ in range(REPEAT) for g in range(GROUPS + 1)]:
                if gg < GROUPS:
                    l1(gg, 0)
                    l1(gg, 1)
                if gg >= 1:
                    l3(gg - 1, 0)
                    l3(gg - 1, 1)
                if gg < GROUPS:
                    l2(gg, 0)
                    l2(gg, 1)
                if gg >= 1:
                    l4(gg - 1, 0)
                    l4(gg - 1, 1)

    nc.compile()
    return nc


def _prep_in_maps(x, W1, b1, W2, b2, W3, b3, W4, b4):
    import ml_dtypes

    bf16 = ml_dtypes.bfloat16
    x = np.asarray(x, dtype=np.float32)

    wp = np.zeros((128, 388), bf16)
    W1T = np.asarray(W1, np.float32).T  # [6, 128]
    for g in range(GPC):
        wp[32 * g : 32 * g + 6, 0:128] = W1T.astype(bf16)
        wp[32 * g + 6, 0:128] = np.asarray(b1, np.float32).astype(bf16)
    wp[:, 128:256] = np.asarray(W2, np.float32).T.astype(bf16)
    wp[:, 256:384] = np.asarray(W3, np.float32).T.astype(bf16)
    wp[:, 384:388] = np.asarray(W4, np.float32).T.astype(bf16)

    bp = np.zeros((128, 2), np.float32)
    bp[:, 0] = np.asarray(b2, np.float32)
    bp[:, 1] = np.asarray(b3, np.float32)

    in_maps = []
    for c in range(CORES):
        xc = x[c * R : (c + 1) * R]  # [R, 6]
        # xin[xg, g, k, q*CHUNK + j] = xc[((xg*4 + q)*GPC + g)*CHUNK + j, k]
        xr = xc.reshape(GROUPS // 4, 4, GPC, CHUNK, 6).transpose(0, 2, 4, 1, 3)
        xr = xr.reshape(GROUPS // 4, GPC, 6, 4 * CHUNK)
        xi = np.empty((GROUPS // 4, GPC, 7, 4 * CHUNK), bf16)
        xi[:, :, 0:6, :] = xr.astype(bf16)
        xi[:, :, 6, :] = 1.0
        in_maps.append({"xin": xi, "wpack": wp, "bpack": bp})
    return in_maps


def _execute(in_maps, trace=False):
    from concourse.bass_utils import run_bass_kernel_spmd

    if "nc" not in _CACHE:
        _CACHE["nc"] = _build()
    return run_bass_kernel_spmd(
        _CACHE["nc"], in_maps, list(range(CORES)), trace=trace
    )


def bench(in_maps, iters=20):
    """Measure the per-iteration device-side execution time of the kernel.

    The NeuronCores are reached through an axon tunnel whose host<->device
    round-trip latency is ~60 ms — three orders of magnitude above the
    kernel itself — so timing one synchronous dispatch measures the
    network, not the hardware.  Instead we enqueue N dispatches
    back-to-back (device-resident inputs, one final block_until_ready) so
    consecutive NEFF executions pipeline on-device, and recover the
    marginal per-iteration cost as the slope between a short and a long
    pipelined batch: slope = (T(N2) - T(N1)) / (N2 - N1).  The one-time
    tunnel round trip cancels in the difference.  Batches are repeated
    interleaved and min-aggregated to reject one-sided scheduling noise.

    Returns [slope_seconds] (list, for min() compatibility).
    """
    import time

    import jax
    from jax.experimental.shard_map import shard_map
    from jax.sharding import Mesh, NamedSharding, PartitionSpec

    import concourse.mybir as mybir
    from concourse import bass2jax

    if "nc" not in _CACHE:
        _CACHE["nc"] = _build()
    nc = _CACHE["nc"]
    bass2jax.install_neuronx_cc_hook()

    in_names, out_names, out_avals = [], [], []
    for alloc in nc.m.functions[0].allocations:
        if not isinstance(alloc, mybir.MemoryLocationSet):
            continue
        name = alloc.memorylocations[0].name
        pid = nc.partition_id_tensor.name if nc.partition_id_tensor else None
        if alloc.kind == "ExternalInput":
            if name != pid:
                in_names.append(name)
        elif alloc.kind == "ExternalOutput":
            out_names.append(name)
            out_avals.append(
                jax.core.ShapedArray(
                    tuple(alloc.tensor_shape), mybir.dt.np(alloc.dtype)
                )
            )
    n_params = len(in_names)
    all_names = tuple(in_names + out_names)

    def _body(*args):
        operands = list(args)
        if nc.partition_id_tensor is not None:
            operands.append(bass2jax.partition_id_tensor())
        outs = bass2jax._bass_exec_p.bind(
            *operands,
            out_avals=tuple(out_avals),
            in_names=all_names
            + ((nc.partition_id_tensor.name,) if nc.partition_id_tensor else ()),
            out_names=tuple(out_names),
            lowering_input_output_aliases=(),
            sim_require_finite=True,
            sim_require_nnan=True,
            nc=nc,
        )
        return tuple(outs)

    devices = jax.devices()[:CORES]
    mesh = Mesh(np.asarray(devices), ("core",))
    in_specs = (PartitionSpec("core"),) * (n_params + len(out_names))
    out_specs = (PartitionSpec("core"),) * len(out_names)
    sm = shard_map(
        _body, mesh=mesh, in_specs=in_specs, out_specs=out_specs, check_rep=False
    )

    concat_in = [
        np.concatenate([np.asarray(in_maps[c][n]) for c in range(CORES)], axis=0)
        for n in in_names
    ]
    zeros = [
        np.zeros((CORES * av.shape[0], *av.shape[1:]), av.dtype) for av in out_avals
    ]
    sh = NamedSharding(mesh, PartitionSpec("core"))
    dev_in = [jax.device_put(a, sh) for a in concat_in]
    dev_zeros = [jax.device_put(z, sh) for z in zeros]

    fn = bass2jax.fast_dispatch_compile(
        lambda: jax.jit(sm, keep_unused=True).lower(*dev_in, *dev_zeros).compile()
    )

    def batch(n):
        t0 = time.perf_counter()
        out = None
        for _ in range(n):
            out = fn(*dev_in, *dev_zeros)
        jax.block_until_ready(out)
        return time.perf_counter() - t0

    batch(2)  # warmup
    n1, n2, reps = 10, 100, max(8, iters // 3)
    t1s, t2s = [], []
    for _ in range(reps):
        t1s.append(batch(n1))
        t2s.append(batch(n2))
    slope = (min(t2s) - min(t1s)) / (n2 - n1)
    slope = max(slope, 1e-9)
    print(
        f"bench: T({n1}) {[round(t * 1e3, 2) for t in t1s]} ms, "
        f"T({n2}) {[round(t * 1e3, 2) for t in t2s]} ms"
    )
    return [slope]


def kernel(**inputs):
    in_maps = _prep_in_maps(
        inputs["x"],
        inputs["W1"],
        inputs["b1"],
        inputs["W2"],
        inputs["b2"],
        inputs["W3"],
        inputs["b3"],
        inputs["W4"],
        inputs["b4"],
    )
    results = _execute(in_maps).results
    outs = []
    for c in range(CORES):
        # yout dims: (xg, half, p, (q, s4, k)); group = xg*4 + q,
        # row = group*2048 + half*1024 + s4*128 + p
        yo = (
            np.asarray(results[c]["yout"])
            .astype(np.float32)
            .reshape(GROUPS // 4, 2, 128, 4, 8, 4)
        )
        outs.append(yo.transpose(0, 3, 1, 4, 2, 5).reshape(R, 4))
    y = np.concatenate(outs, axis=0)
    y += np.asarray(inputs["b4"], np.float32)  # layer-4 bias, added on host
    return np.ascontiguousarray(y.astype(np.float32))


# revision 12
# speedup vs baseline: 1.0758x; 1.0185x over previous
"""Trainium2 Bass kernel for a 4-layer NeRF-style MLP.

    y = relu(relu(relu(x@W1.T+b1)@W2.T+b2)@W3.T+b3)@W4.T+b4
    x: [1048576, 6] fp32 -> y: [1048576, 4] fp32

Strategy: pure data parallel over 8 NeuronCores (131072 rows each).
On-device layout keeps features on SBUF partitions and rows on the free
dim, so every layer's PSUM output is directly the next layer's matmul
rhs -- no transposes anywhere.

The entire on-device datapath is bf16 (x, W1..W4, h1..h3): every
LDWEIGHTS gets Fast Weight Load (2 cols/cycle), the moving operands
halve their SBUF read bandwidth, and the PE instruction stream is dense
enough to keep the HAM clock-gate at K=8/8 (2.4 GHz).  PSUM stays f32.

Per core, rows are processed in groups of 4 chunks x 512 rows, split
into two independent half-group chains (half 0: chunks 0-1, half 1:
chunks 2-3), each owning a 2-bank PSUM tile.  With pool bufs=2, four
half-chains are in flight; the software pipeline is interleaved at
half-group granularity (L1(g,0) L1(g,1) L3(g-1,0) L3(g-1,1) L2(g,0)
L2(g,1) L4(g-1,0) L4(g-1,1)) so every eviction wait is covered by
another chain's matmuls.

  - layer 1 (K=6+1): the 4 chunks are packed into the four 32-row PE
    groups (tile_position row packing) and run concurrently; the bias is
    folded into the matmul via a constant ones-row in x (K=7).
  - layers 2/3 (K=128): one matmul per chunk, bf16.
  - layer 4 is computed transposed (h3-slice stationary, W4.T moving,
    N=4); the group's output is a dense [128, 32] PSUM block per half,
    so its eviction is nearly free.  b4 is added on the host.
  - evictions are fused bias+ReLU ops; the half-chain <-> ScalarE/VectorE
    assignment alternates by group parity (pinning layers to engines
    measured ~9% slower: per-engine bursts serialize the chains).
  - one dummy "heater" matmul per group (into the just-evicted PSUM
    region) keeps the PE's HAM clock-gate at K=8/8: without it the PE
    idle-dips during the LDW-heavy L4 phase re-throttle the array to
    1.2 GHz for 40-60% of the kernel.
  - all weights ship as one packed [128, 388] bf16 input plus a tiny
    [128, 2] f32 bias input.
"""

import numpy as np

N = 1048576
CORES = 8
R = N // CORES            # rows per core
CHUNK = 512               # rows per matmul (one PSUM bank of fp32)
GPC = 4                   # chunks per group
GROUPS = R // (CHUNK * GPC)   # 64
GW = GPC * CHUNK          # 2048 columns per group
REPEAT = 1                # times to run the whole compute body (bench only)
HEATER = True             # keep-HAM-warm dummy matmul per group

_CACHE = {}


def _build():
    import concourse.bacc as bacc
    import concourse.mybir as mybir
    import concourse.tile as tile
    from concourse.compiler_utils import get_compiler_flags, set_compiler_flags

    # The environment's default backend options ship --enable-ldw-opt=false
    # (a blanket workaround for an FWL hang that only affects fp32-HI
    # matmuls).  Every matmul in this kernel is bf16, so Fast Weight Load
    # is safe -- and halves every LDWEIGHTS (128 cols: ~107ns -> ~55ns).
    set_compiler_flags(
        [
            f.replace("--enable-ldw-opt=false", "--enable-ldw-opt=true")
            for f in get_compiler_flags()
        ]
    )

    f32 = mybir.dt.float32
    bf16 = mybir.dt.bfloat16
    Relu = mybir.ActivationFunctionType.Relu
    op_add = mybir.AluOpType.add
    op_max = mybir.AluOpType.max

    nc = bacc.Bacc("TRN2", target_bir_lowering=False, debug=False)

    xin = nc.dram_tensor(
        "xin", [GROUPS // 4, GPC, 7, 4 * CHUNK], bf16, kind="ExternalInput"
    ).ap()
    wpack = nc.dram_tensor(
        "wpack", [128, 388], bf16, kind="ExternalInput"
    ).ap()  # w1(+b1) | w2 | w3 | w4
    bpack = nc.dram_tensor(
        "bpack", [128, 2], f32, kind="ExternalInput"
    ).ap()  # b2 | b3
    yout = nc.dram_tensor(
        "yout", [GROUPS // 4, 2, 128, 128], bf16, kind="ExternalOutput"
    ).ap()

    with tile.TileContext(nc) as tc:
        with (
            tc.tile_pool(name="const", bufs=1) as cpool,
            tc.tile_pool(name="x", bufs=4) as xpool,
            tc.tile_pool(name="h", bufs=6) as hpool,
            tc.tile_pool(name="o", bufs=4) as opool,
            tc.tile_pool(name="psum", bufs=2, space="PSUM") as ppool,
        ):
            wps = cpool.tile([128, 388], bf16, tag="wp")
            nc.sync.dma_start(out=wps[:], in_=wpack)
            bps = cpool.tile([128, 2], f32, tag="bp")
            nc.sync.dma_start(out=bps[:], in_=bpack)
            w1s = wps[:, 0:128]
            w2s = wps[:, 128:256]
            w3s = wps[:, 256:384]
            w4s = wps[:, 384:388]
            b2s = bps[:, 0:1]
            b3s = bps[:, 1:2]

            w1r = w1s.rearrange("(a b) c -> a b c", b=32)

            HW = GW // 2  # 1024 columns: each half-group (2 chunks)
            st = {}       # per-group in-flight tiles
            xts = {}      # x tile per 4-group block
            oab = {}      # output accumulation tiles per 4-group block

            def evict_relu(use_act, out_ap, in_ap, bias_ap):
                """bias+ReLU PSUM->SBUF eviction on either engine."""
                if use_act:
                    if bias_ap is None:
                        nc.scalar.activation(out_ap, in_ap, Relu)
                    else:
                        nc.scalar.activation(out_ap, in_ap, Relu, bias=bias_ap)
                elif bias_ap is None:
                    nc.vector.tensor_scalar(
                        out=out_ap,
                        in0=in_ap,
                        scalar1=0.0,
                        scalar2=None,
                        op0=op_max,
                    )
                else:
                    nc.vector.tensor_scalar(
                        out=out_ap,
                        in0=in_ap,
                        scalar1=bias_ap,
                        scalar2=0.0,
                        op0=op_add,
                        op1=op_max,
                    )

            def evict_copy(use_act, out_ap, in_ap):
                if use_act:
                    nc.scalar.activation(
                        out_ap, in_ap, mybir.ActivationFunctionType.Copy
                    )
                else:
                    nc.vector.tensor_copy(out=out_ap, in_=in_ap)

            def load_x(blk):
                """DMA one 4-group block of x into SBUF."""
                if blk >= GROUPS // 4 or blk in xts:
                    return
                xt = xpool.tile([128, 4 * CHUNK], bf16, tag="x")
                xtr = xt.rearrange("(a b) c -> a b c", b=32)
                for c in range(GPC):
                    nc.sync.dma_start(out=xtr[c, 0:7, :], in_=xin[blk, c])
                xts[blk] = xtr

            def use_act(g, half):
                # alternate engine<->half-chain by group parity to balance
                return (g % 2 == 0) == (half == 0)

            def l1(g, half):
                """layer-1 matmuls + L1 eviction for one half-group (x is
                prefetched a block ahead so L1 never waits on the DMA)."""
                q = g % 4
                if half == 0 and q == 0:
                    load_x(g // 4)      # no-op except for block 0
                    load_x(g // 4 + 1)  # prefetch next block
                xtr = xts[g // 4]
                p = ppool.tile([128, HW], f32, tag=f"p{half}")
                for cc in range(2):
                    c = 2 * half + cc
                    nc.tensor.matmul(
                        p[:, cc * CHUNK : (cc + 1) * CHUNK],
                        lhsT=w1r[c, 0:7, :],
                        rhs=xtr[c, 0:7, q * CHUNK : (q + 1) * CHUNK],
                        start=True,
                        stop=True,
                        tile_position=(32 * c, 0),
                    )
                h = hpool.tile([128, HW], bf16, tag=f"h1{half}")
                evict_relu(use_act(g, half), h[:, :], p[:, :], None)
                st[(g, half)] = {"p": p, "h1": h}

            def l2(g, half):
                s = st[(g, half)]
                for cc in range(2):
                    nc.tensor.matmul(
                        s["p"][:, cc * CHUNK : (cc + 1) * CHUNK],
                        lhsT=w2s,
                        rhs=s["h1"][:, cc * CHUNK : (cc + 1) * CHUNK],
                        start=True,
                        stop=True,
                    )
                h = hpool.tile([128, HW], bf16, tag=f"h2{half}")
                evict_relu(use_act(g, half), h[:, :], s["p"][:, :], b2s)
                s["h2"] = h

            def l3(g, half):
                s = st[(g, half)]
                for cc in range(2):
                    nc.tensor.matmul(
                        s["p"][:, cc * CHUNK : (cc + 1) * CHUNK],
                        lhsT=w3s,
                        rhs=s["h2"][:, cc * CHUNK : (cc + 1) * CHUNK],
                        start=True,
                        stop=True,
                    )
                h = hpool.tile([128, HW], bf16, tag=f"h3{half}")
                evict_relu(use_act(g, half), h[:, :], s["p"][:, :], b3s)
                s["h3"] = h

            def l4(g, half):
                """layer 4 (transposed, bf16 FWL), output copy + DMA."""
                s = st.pop((g, half))
                q = g % 4
                for sl in range(8):
                    nc.tensor.matmul(
                        s["p"][:, 4 * sl : 4 * sl + 4],
                        lhsT=s["h3"][:, 128 * sl : 128 * sl + 128],
                        rhs=w4s[:, :],
                        start=True,
                        stop=True,
                        skip_group_check=True,
                    )
                if q == 0 and half == 0:
                    ota = opool.tile([128, 128], bf16, tag="oa")
                    otb = opool.tile([128, 128], bf16, tag="ob")
                    oab[g // 4] = (ota, otb)
                ot = oab[g // 4][half]
                evict_copy(use_act(g, half), ot[:, 32 * q : 32 * q + 32], s["p"][:, 0:32])
                if HEATER and half == 1:
                    # dummy matmul into the already-evicted region of this
                    # PSUM tile: real PE-array streaming activity in the
                    # otherwise LDW-heavy L4 phase, keeping the HAM
                    # clock-gate at K=8/8.  Output is garbage, never read;
                    # the tile recycles to L1 right after (WAW-serialized).
                    nc.tensor.matmul(
                        s["p"][:, 0:384],
                        lhsT=w2s,
                        rhs=wps[:, 0:384],
                        start=True,
                        stop=True,
                        skip_group_check=True,
                    )
                if q == 3 and half == 1:
                    ota, otb = oab[g // 4]
                    nc.sync.dma_start(out=yout[g // 4, 0], in_=ota[:])
                    nc.sync.dma_start(out=yout[g // 4, 1], in_=otb[:])
                    del oab[g // 4], xts[g // 4]

            # software pipeline over groups at half-group granularity: the
            # independent half-chains give the scheduler material to fill
            # every eviction wait with another chain's matmuls.
            for gg in [g for _ in range(REPEAT) for g in range(GROUPS + 1)]:
                if gg < GROUPS:
                    l1(gg, 0)
                    l1(gg, 1)
                if gg >= 1:
                    l3(gg - 1, 0)
                    l3(gg - 1, 1)
                if gg < GROUPS:
                    l2(gg, 0)
                    l2(gg, 1)
                if gg >= 1:
                    l4(gg - 1, 0)
                    l4(gg - 1, 1)

    nc.compile()
    return nc


def _prep_in_maps(x, W1, b1, W2, b2, W3, b3, W4, b4):
    import ml_dtypes

    bf16 = ml_dtypes.bfloat16
    x = np.asarray(x, dtype=np.float32)

    wp = np.zeros((128, 388), bf16)
    W1T = np.asarray(W1, np.float32).T  # [6, 128]
    for g in range(GPC):
        wp[32 * g : 32 * g + 6, 0:128] = W1T.astype(bf16)
        wp[32 * g + 6, 0:128] = np.asarray(b1, np.float32).astype(bf16)
    wp[:, 128:256] = np.asarray(W2, np.float32).T.astype(bf16)
    wp[:, 256:384] = np.asarray(W3, np.float32).T.astype(bf16)
    wp[:, 384:388] = np.asarray(W4, np.float32).T.astype(bf16)

    bp = np.zeros((128, 2), np.float32)
    bp[:, 0] = np.asarray(b2, np.float32)
    bp[:, 1] = np.asarray(b3, np.float32)

    in_maps = []
    for c in range(CORES):
        xc = x[c * R : (c + 1) * R]  # [R, 6]
        # xin[xg, g, k, q*CHUNK + j] = xc[((xg*4 + q)*GPC + g)*CHUNK + j, k]
        xr = xc.reshape(GROUPS // 4, 4, GPC, CHUNK, 6).transpose(0, 2, 4, 1, 3)
        xr = xr.reshape(GROUPS // 4, GPC, 6, 4 * CHUNK)
        xi = np.empty((GROUPS // 4, GPC, 7, 4 * CHUNK), bf16)
        xi[:, :, 0:6, :] = xr.astype(bf16)
        xi[:, :, 6, :] = 1.0
        in_maps.append({"xin": xi, "wpack": wp, "bpack": bp})
    return in_maps


def _execute(in_maps, trace=False):
    from concourse.bass_utils import run_bass_kernel_spmd

    if "nc" not in _CACHE:
        _CACHE["nc"] = _build()
    return run_bass_kernel_spmd(
        _CACHE["nc"], in_maps, list(range(CORES)), trace=trace
    )


def bench(in_maps, iters=20):
    """Measure the per-iteration device-side execution time of the kernel.

    The NeuronCores are reached through an axon tunnel whose host<->device
    round-trip latency is ~60 ms — three orders of magnitude above the
    kernel itself — so timing one synchronous dispatch measures the
    network, not the hardware.  Instead we enqueue N dispatches
    back-to-back (device-resident inputs, one final block_until_ready) so
    consecutive NEFF executions pipeline on-device, and recover the
    marginal per-iteration cost as the slope between a short and a long
    pipelined batch: slope = (T(N2) - T(N1)) / (N2 - N1).  The one-time
    tunnel round trip cancels in the difference.  Batches are repeated
    interleaved and min-aggregated to reject one-sided scheduling noise.

    Returns [slope_seconds] (list, for min() compatibility).
    """
    import time

    import jax
    from jax.experimental.shard_map import shard_map
    from jax.sharding import Mesh, NamedSharding, PartitionSpec

    import concourse.mybir as mybir
    from concourse import bass2jax

    if "nc" not in _CACHE:
        _CACHE["nc"] = _build()
    nc = _CACHE["nc"]
    bass2jax.install_neuronx_cc_hook()

    in_names, out_names, out_avals = [], [], []
    for alloc in nc.m.functions[0].allocations:
        if not isinstance(alloc, mybir.MemoryLocationSet):
            continue
        name = alloc.memorylocations[0].name
        pid = nc.partition_id_tensor.name if nc.partition_id_tensor else None
        if alloc.kind == "ExternalInput":
            if name != pid:
                in_names.append(name)
        elif alloc.kind == "ExternalOutput":
            out_names.append(name)
            out_avals.append(
                jax.core.ShapedArray(
                    tuple(alloc.tensor_shape), mybir.dt.np(alloc.dtype)
                )
            )
    n_params = len(in_names)
    all_names = tuple(in_names + out_names)

    def _body(*args):
        operands = list(args)
        if nc.partition_id_tensor is not None:
            operands.append(bass2jax.partition_id_tensor())
        outs = bass2jax._bass_exec_p.bind(
            *operands,
            out_avals=tuple(out_avals),
            in_names=all_names
            + ((nc.partition_id_tensor.name,) if nc.partition_id_tensor else ()),
            out_names=tuple(out_names),
            lowering_input_output_aliases=(),
            sim_require_finite=True,
            sim_require_nnan=True,
            nc=nc,
        )
        return tuple(outs)

    devices = jax.devices()[:CORES]
    mesh = Mesh(np.asarray(devices), ("core",))
    in_specs = (PartitionSpec("core"),) * (n_params + len(out_names))
    out_specs = (PartitionSpec("core"),) * len(out_names)
    sm = shard_map(
        _body, mesh=mesh, in_specs=in_specs, out_specs=out_specs, check_rep=False
    )

    concat_in = [
        np.concatenate([np.asarray(in_maps[c][n]) for c in range(CORES)], axis=0)
        for n in in_names
    ]
    zeros = [
        np.zeros((CORES * av.shape[0], *av.shape[1:]), av.dtype) for av in out_avals
    ]
    sh = NamedSharding(mesh, PartitionSpec("core"))
    dev_in = [jax.device_put(a, sh) for a in concat_in]
    dev_zeros = [jax.device_put(z, sh) for z in zeros]

    fn = bass2jax.fast_dispatch_compile(
        lambda: jax.jit(sm, keep_unused=True).lower(*dev_in, *dev_zeros).compile()
    )

    def batch(n):
        t0 = time.perf_counter()
        out = None
        for _ in range(n):
            out = fn(*dev_in, *dev_zeros)
        jax.block_until_ready(out)
        return time.perf_counter() - t0

    batch(2)  # warmup
    n1, n2, reps = 10, 100, max(8, iters // 3)
    t1s, t2s = [], []
    for _ in range(reps):
        t1s.append(batch(n1))
        t2s.append(batch(n2))
    slope = (min(t2s) - min(t1s)) / (n2 - n1)
    slope = max(slope, 1e-9)
    print(
        f"bench: T({n1}) {[round(t * 1e3, 2) for t in t1s]} ms, "
        f"T({n2}) {[round(t * 1e3, 2) for t in t2s]} ms"
    )
    return [slope]


def kernel(**inputs):
    in_maps = _prep_in_maps(
        inputs["x"],
        inputs["W1"],
        inputs["b1"],
        inputs["W2"],
        inputs["b2"],
        inputs["W3"],
        inputs["b3"],
        inputs["W4"],
        inputs["b4"],
    )
    results = _execute(in_maps).results
    outs = []
    for c in range(CORES):
        # yout dims: (xg, half, p, (q, s4, k)); group = xg*4 + q,
        # row = group*2048 + half*1024 + s4*128 + p
        yo = (
            np.asarray(results[c]["yout"])
            .astype(np.float32)
            .reshape(GROUPS // 4, 2, 128, 4, 8, 4)
        )
        outs.append(yo.transpose(0, 3, 1, 4, 2, 5).reshape(R, 4))
    y = np.concatenate(outs, axis=0)
    y += np.asarray(inputs["b4"], np.float32)  # layer-4 bias, added on host
    return np.ascontiguousarray(y.astype(np.float32))
